# revision 38
# baseline (speedup 1.0000x reference)
"""Trainium2 Bass kernel for nn_Attention (llama-style attention layer, fp32).

Full inputs in, full output out. 8-way tensor-parallel over heads (4 heads
per core, both batches on every core), bf16 matmul operands (fp32 PSUM):
  - merged q/k/v projections, 512-token stripes, full-32 k-tile PSUM
    accumulation; RoPE fused into q/k eviction (pair-swap via DVE
    stream_shuffle, so the PE only does real GEMM work)
  - v projected directly in [token, feature] layout (x as the stationary
    operand) so attention needs no transposes
  - per-(chunk, head) attention in [feat, tok] layout; softmax denominator
    via DVE tree-adds + one small all-ones matmul; normalization on eviction
  - output projection with row-sharded wo (SBUF-resident, 4MB): each core
    computes a partial y for ALL tokens over its own 512 features, then a
    per-token-chunk ReduceScatter(add) over the 8 cores delivers the final
    [512 dout, 512 tok] shard; collectives pipeline behind compute
  - two HWDGE rings: loads stream on the SP(sync) ring, all DRAM stores go
    out on the Activation ring so stores never head-of-line-block prefetch
    and prefetch never delays eviction drains
"""
import sys

sys.path.insert(0, "/opt/trn_rl_repo")

import numpy as np
import ml_dtypes

import concourse.bass as bass
import concourse.mybir as mybir
import concourse.tile as tile
from concourse import bacc
from concourse.bass import ds, ts
from concourse.bass_utils import run_bass_kernel_spmd

DIM = 4096
N_HEADS = 32
HEAD_DIM = 128
B, S = 2, 2048
TOK = B * S                   # 4096 global tokens
N_CORES = 8
HPC = N_HEADS // N_CORES      # heads per core = 4
FPC = HPC * HEAD_DIM          # features per core = 512
P = 128
KO = DIM // P                 # 32 k-tiles over the model dim
NSTRIPE = 8                   # 512-token projection stripes
NCHUNK = 8                    # 512-token attention/output chunks
SCALE = 1.0 / float(np.sqrt(HEAD_DIM))

f32 = mybir.dt.float32
bf16 = mybir.dt.bfloat16
EXP = mybir.ActivationFunctionType.Exp
COPY = mybir.ActivationFunctionType.Copy
MULT = mybir.AluOpType.mult
ADD = mybir.AluOpType.add
SHUF = [i ^ 1 for i in range(32)]    # pair swap within each 32-lane group

_CACHE = {}


def _build():
    nc = bacc.Bacc(
        "TRN2", target_bir_lowering=False, debug=False, num_devices=N_CORES
    )

    xT = nc.dram_tensor("xT", [DIM, TOK], bf16, kind="ExternalInput")
    wqH = nc.dram_tensor("wqH", [HPC, P, KO, P], bf16, kind="ExternalInput")
    wkH = nc.dram_tensor("wkH", [HPC, P, KO, P], bf16, kind="ExternalInput")
    wvH = nc.dram_tensor("wvH", [P, KO, FPC], bf16, kind="ExternalInput")
    # wo rows for this core's features: [p_feat, j, nt, dout]
    woH = nc.dram_tensor("woH", [P, HPC, DIM // P, P], bf16,
                         kind="ExternalInput")
    cb_d = nc.dram_tensor("cb", [P, S], bf16, kind="ExternalInput")
    ss_d = nc.dram_tensor("ss", [P, S], bf16, kind="ExternalInput")
    ones_d = nc.dram_tensor("ones", [P, P], bf16, kind="ExternalInput")
    # this core's dout rows (512c..512c+512) for all tokens. bf16: the data
    # is already bf16 after the ReduceScatter, so storing bf16 loses nothing,
    # halves the copy traffic, and (no dtype cast) lets the OT->out copies
    # ride the sync HWDGE ring instead of jamming the gpsimd store queue.
    out_e = nc.dram_tensor("out", [FPC, TOK], bf16, kind="ExternalOutput")

    xT3 = xT.ap().rearrange("(ko p) t -> p ko t", p=P)       # [128, 32, 4096]
    oe3 = out_e.ap().rearrange("(dt p) t -> p dt t", p=P)    # [128, 4, 4096]

    with tile.TileContext(nc) as tc:
        # flat PSUM pools shared by every phase: 3*2 + 2*1 = 8 banks.
        # Scores get 3 double-bank tiles so the exp eviction stream can lag
        # up to ~3.3us behind the score matmuls without blocking the PE;
        # the accumulation chains only ever need 2-deep rotation.
        with tc.tile_pool(name="psA", bufs=3, space="PSUM") as ppA, \
             tc.tile_pool(name="psB", bufs=2, space="PSUM") as ppB, \
             tc.tile_pool(name="dram", bufs=1, space="DRAM") as drp, \
             tc.tile_pool(name="const", bufs=1) as constp, \
             tc.tile_pool(name="kh", bufs=8) as khp, \
             tc.tile_pool(name="vh", bufs=8) as vhp, \
             tc.tile_pool(name="qh", bufs=6) as qhp:
            # DRAM staging, split per chunk/batch for fine-grained deps
            qd = [drp.tile([HPC, P, 512], bf16, tag=f"qd{t}", name=f"qd{t}")
                  for t in range(NCHUNK)]
            kd = [drp.tile([HPC, P, S], bf16, tag=f"kd{b}", name=f"kd{b}")
                  for b in range(B)]
            # v: [p_tok, tile, (m q)] per batch; written [p,512] per tok-tile
            vd = [drp.tile([P, S // P, FPC], bf16, tag=f"vd{b}",
                           name=f"vd{b}") for b in range(B)]
            BT = [drp.tile([N_CORES, FPC, 512], bf16, tag=f"bt{t}",
                           name=f"bt{t}") for t in range(NCHUNK - 1)]
            OT = [drp.tile([FPC, 512], bf16, tag=f"ot{t}", name=f"ot{t}")
                  for t in range(NCHUNK - 1)]
            # first and last chunks split their RS into two halves of the
            # shard rows (keyed by nt%4): chunk 0's first half starts the
            # serialized collective stream ~16us earlier, and chunk 7's
            # first half overlaps the tail of the wo block. Finer splits
            # lose: small RDH transfers are inefficient.
            LQ = [(0, 2), (2, 4)]              # nt%4 ranges per piece
            BTh = [drp.tile([N_CORES, P * (q1 - q0), 512], bf16,
                            tag=f"bth{h}", name=f"bth{h}")
                   for h, (q0, q1) in enumerate(LQ)]
            OTh = [drp.tile([P * (q1 - q0), 512], bf16, tag=f"oth{h}",
                            name=f"oth{h}") for h, (q0, q1) in enumerate(LQ)]
            BTz = [drp.tile([N_CORES, P * (q1 - q0), 512], bf16,
                            tag=f"btz{h}", name=f"btz{h}")
                   for h, (q0, q1) in enumerate(LQ)]
            OTz = [drp.tile([P * (q1 - q0), 512], bf16, tag=f"otz{h}",
                            name=f"otz{h}") for h, (q0, q1) in enumerate(LQ)]
            # tiny warm-up collectives: absorb the cold-start of the first
            # ReduceScatter and keep the SDMA collective path warm through
            # phase 1 (the later ones are keyed off qd tiles so they fire
            # mid-phase-1 instead of all at the start; the qd->wui copies ride
            # the gpsimd queue so their waits never block the load ring)
            win = drp.tile([N_CORES, P, 8], bf16, tag="win", name="win")
            wout = drp.tile([P, 8], bf16, tag="wout", name="wout")
            nc.gpsimd.collective_compute(
                "ReduceScatter",
                mybir.AluOpType.add,
                replica_groups=[list(range(N_CORES))],
                ins=[win[:]],
                outs=[wout[:]],
            )
            wui = [drp.tile([N_CORES, 64], bf16, tag=f"wui{i}",
                            name=f"wui{i}") for i in range(2)]
            wuo = [drp.tile([64], bf16, tag=f"wuo{i}", name=f"wuo{i}")
                   for i in range(2)]
            # the last warm-up is 512KB: the 1KB ones don't exercise the
            # RDH fold engines/credits, so the first real 4MB RS still paid
            # a ~16us cold penalty that cascades through the saturated
            # collective stream
            wu3i = drp.tile([N_CORES, P, 256], bf16, tag="wu3i", name="wu3i")
            wu3o = drp.tile([P, 256], bf16, tag="wu3o", name="wu3o")

            ones_sb = constp.tile([P, P], bf16, tag="ones", name="ones_sb")
            cb_sb = constp.tile([P, S], bf16, tag="cb", name="cb_sb")
            ss_sb = constp.tile([P, S], bf16, tag="ss", name="ss_sb")

            # ---------- Phase 1: q/k/v projections (+RoPE on q,k) ----------
            with tc.tile_pool(name="p1_x", bufs=44) as xp, \
                 tc.tile_pool(name="p1_w", bufs=2) as wtp, \
                 tc.tile_pool(name="p1_wv", bufs=1) as wvp, \
                 tc.tile_pool(name="p1_t", bufs=3) as rp, \
                 tc.tile_pool(name="p1_v", bufs=4) as vtp:
                wv_sb = wvp.tile([P, KO, FPC], bf16, tag="wv", name="wv_sb")

                for n in range(NSTRIPE):
                    b, nl = divmod(n, NSTRIPE // B)
                    tok0 = 512 * n
                    rtok = tok0 % S
                    xs = [xp.tile([P, 512], bf16, tag="xsl", name="xs")
                          for _ in range(KO)]
                    wts = {}
                    if n == 0:
                        # startup-critical ordering: the very first q chain
                        # needs xs[0..] and head-0 q weights; everything else
                        # (cos/sin, wv, ones) is needed later and must not
                        # delay the first matmul
                        nc.sync.dma_start(xs[0][:], xT3[:, 0, ds(tok0, 512)])
                        wta = wtp.tile([P, KO // 2, P], bf16, tag="wta",
                                       name="wta")
                        nc.sync.dma_start(wta[:], wqH.ap()[0][:, 0:KO // 2, :])
                        for ko in range(1, 8):
                            nc.sync.dma_start(xs[ko][:],
                                              xT3[:, ko, ds(tok0, 512)])
                        wtb = wtp.tile([P, KO // 2, P], bf16, tag="wtb",
                                       name="wtb")
                        nc.sync.dma_start(wtb[:],
                                          wqH.ap()[0][:, KO // 2:KO, :])
                        wts[(0, 0)] = (wta, wtb)
                        nc.sync.dma_start(cb_sb[:], cb_d.ap())
                        nc.sync.dma_start(ss_sb[:], ss_d.ap())
                        for ko in range(8, KO):
                            nc.sync.dma_start(xs[ko][:],
                                              xT3[:, ko, ds(tok0, 512)])
                    else:
                        # head-0 q weights ahead of the x block: the x tile
                        # buffers only free as the previous stripe's v chains
                        # retire, and the first chain must not also wait for
                        # weights queued behind that
                        wta = wtp.tile([P, KO // 2, P], bf16, tag="wta",
                                       name="wta")
                        nc.sync.dma_start(wta[:], wqH.ap()[0][:, 0:KO // 2, :])
                        wtb = wtp.tile([P, KO // 2, P], bf16, tag="wtb",
                                       name="wtb")
                        nc.sync.dma_start(wtb[:],
                                          wqH.ap()[0][:, KO // 2:KO, :])
                        wts[(0, 0)] = (wta, wtb)
                        for ko in range(KO):
                            nc.sync.dma_start(xs[ko][:],
                                              xT3[:, ko, ds(tok0, 512)])
                    # q and k projections with fused RoPE; weights land as
                    # two half-tiles so the accumulation chain starts after
                    # only the first half arrives
                    for pi, wH in ((0, wqH), (1, wkH)):
                        for m in range(HPC):
                            if (pi, m) in wts:
                                wta, wtb = wts[(pi, m)]
                            else:
                                wta = wtp.tile([P, KO // 2, P], bf16,
                                               tag="wta", name="wta")
                                nc.sync.dma_start(wta[:],
                                                  wH.ap()[m][:, 0:KO // 2, :])
                                wtb = wtp.tile([P, KO // 2, P], bf16,
                                               tag="wtb", name="wtb")
                                nc.sync.dma_start(wtb[:],
                                                  wH.ap()[m][:, KO // 2:KO, :])
                            ps = ppB.tile([P, 512], f32, tag="sm", name="ps")
                            for ko in range(KO):
                                wt_half = wta if ko < KO // 2 else wtb
                                nc.tensor.matmul(
                                    ps[:], wt_half[:, ko % (KO // 2)],
                                    xs[ko][:],
                                    start=(ko == 0), stop=(ko == KO - 1),
                                )
                            raw = rp.tile([P, 512], bf16, tag="raw",
                                          name="raw")
                            nc.scalar.activation(raw[:], ps[:], COPY)
                            sw = rp.tile([P, 512], bf16, tag="sw", name="sw")
                            nc.vector.stream_shuffle(sw[:], raw[:], SHUF)
                            t1 = rp.tile([P, 512], bf16, tag="t1", name="t1")
                            nc.vector.tensor_tensor(
                                t1[:], raw[:], cb_sb[:, ds(rtok, 512)], MULT
                            )
                            t2 = rp.tile([P, 512], bf16, tag="t2", name="t2")
                            nc.vector.tensor_tensor(
                                t2[:], sw[:], ss_sb[:, ds(rtok, 512)], MULT
                            )
                            qf = rp.tile([P, 512], bf16, tag="qf", name="qf")
                            nc.vector.tensor_tensor(qf[:], t1[:], t2[:], ADD)
                            # stores ride the gpsimd SWDGE: the sync ring must
                            # stream loads without store-waits, and a
                            # DMA_DIRECT2D costs ~0.7us of issuing-engine time
                            # that Scalar/Vector can't spare
                            if pi == 0:
                                nc.gpsimd.dma_start(qd[n][m], qf[:])
                            else:
                                nc.gpsimd.dma_start(
                                    kd[b][m][:, ds(512 * nl, 512)], qf[:]
                                )
                        if n == 0 and pi == 1:
                            # v weights are first needed ~70us in; ones only
                            # in phase 2. Issue behind ALL stripe-0 q/k
                            # weights so the k chains never wait behind the
                            # 4MB wv transfer.
                            nc.sync.dma_start(wv_sb[:], wvH.ap())
                            nc.sync.dma_start(ones_sb[:], ones_d.ap())
                    # v projection, direct [tok, feat] layout
                    for tt in range(4):
                        ps_v = ppB.tile([P, 512], f32, tag="sm", name="ps_v")
                        for ko in range(KO):
                            nc.tensor.matmul(
                                ps_v[:], xs[ko][:, ts(tt, P)], wv_sb[:, ko],
                                start=(ko == 0), stop=(ko == KO - 1),
                            )
                        vt = vtp.tile([P, 512], bf16, tag="vt", name="vt")
                        nc.scalar.activation(vt[:], ps_v[:], COPY)
                        nc.gpsimd.dma_start(vd[b][:, 4 * nl + tt, :], vt[:])
                    # staggered collective warm-ups, emitted inline in the
                    # gpsimd stream so they fire ~1/4, ~5/8 and ~7/8 through
                    # phase 1 and keep the SDMA collective path warm right up
                    # to the first chunk's ReduceScatter
                    if n in (1, 5):
                        i = {1: 0, 5: 1}[n]
                        nc.gpsimd.dma_start(wui[i][0][0:64],
                                            qd[n][0][0][0:64])
                        nc.gpsimd.collective_compute(
                            "ReduceScatter",
                            mybir.AluOpType.add,
                            replica_groups=[list(range(N_CORES))],
                            ins=[wui[i][:]],
                            outs=[wuo[i][:]],
                        )
                    elif n == 7:
                        nc.gpsimd.dma_start(wu3i[0][0][0:64],
                                            qd[n][0][0][0:64])
                        nc.gpsimd.collective_compute(
                            "ReduceScatter",
                            mybir.AluOpType.add,
                            replica_groups=[list(range(N_CORES))],
                            ins=[wu3i[:]],
                            outs=[wu3o[:]],
                        )

            # ---------- Phase 2: attention + output projection, chunked ----
            with tc.tile_pool(name="p2_wo", bufs=1) as wop, \
                 tc.tile_pool(name="p2_e", bufs=14) as ep, \
                 tc.tile_pool(name="p2_tr", bufs=5) as trp, \
                 tc.tile_pool(name="p2_dn", bufs=3) as dnp, \
                 tc.tile_pool(name="p2_at", bufs=8) as atp, \
                 tc.tile_pool(name="p2_yb", bufs=24) as ybp:
                woc = wop.tile([P, HPC, DIM // P, P], bf16, tag="woc",
                               name="woc")

                # prefetch k/v/q in dependency order so the load ring
                # streams through phase-1's tail with no stalls. Batch 1's
                # 8MB is deferred into the chunk loop so it doesn't burst
                # concurrently with the first chunks' ReduceScatters (our
                # DMA packets and the collective's share SDMA engine slots).
                khs, vhs, qhs = {}, {}, {}

                def prefetch_kv(b):
                    for m in range(HPC):
                        kh = khp.tile([P, S], bf16, tag="kh", name="kh")
                        nc.sync.dma_start(kh[:], kd[b][m])
                        vh = vhp.tile([P, S // P, P], bf16, tag="vh",
                                      name="vh")
                        nc.sync.dma_start(
                            vh[:],
                            vd[b].rearrange("p tt (m q) -> p tt m q",
                                            m=HPC)[:, :, m, :],
                        )
                        khs[(b, m)] = kh
                        vhs[(b, m)] = vh

                def prefetch_q(b):
                    for qt in range(4):
                        t = 4 * b + qt
                        qh = qhp.tile([P, HPC, 512], bf16, tag="qh",
                                      name="qh")
                        nc.sync.dma_start(
                            qh[:], qd[t][:].rearrange("m p q -> p m q")
                        )
                        qhs[t] = qh

                prefetch_kv(0)
                prefetch_q(0)
                # woc is first needed ~100us into phase 2; keep it behind the
                # batch-0 prefetch so stripe-7's weight loads stay prompt
                nc.sync.dma_start(woc[:], woH.ap())

                # chunk-indexed state so scores can run one chunk ahead of
                # the output projection: issuing scores(t+1, 0..1) before
                # wo(t) gives the Scalar engine the whole wo window of slack
                # for its exp stream instead of just the attention window
                etsm, psom, ats = {}, {}, {}

                def do_scores(t, m):
                    kh = khs[(t // 4, m)]
                    ets = []
                    for k2 in range(S // P // 2):
                        ps_s = ppA.tile([P, 1024], f32, tag="big",
                                        name="ps_s")
                        for kk in range(2):
                            kt = 2 * k2 + kk
                            nc.tensor.matmul(
                                ps_s[:, ts(kk, 512)],
                                kh[:, ts(kt, P)], qhs[t][:, m],
                                start=True, stop=True,
                            )
                        et = ep.tile([P, 1024], bf16, tag="e", name="et")
                        nc.scalar.activation(et[:], ps_s[:], EXP, scale=SCALE)
                        ets.append(et)
                    etsm[(t, m)] = ets

                def do_pv(t, m):
                    vh = vhs[(t // 4, m)]
                    ets = etsm[(t, m)]
                    ps_o = ppB.tile([P, 512], f32, tag="sm", name="ps_o")
                    for kt in range(S // P):
                        nc.tensor.matmul(
                            ps_o[:], vh[:, kt],
                            ets[kt // 2][:, ts(kt % 2, 512)],
                            start=(kt == 0), stop=(kt == S // P - 1),
                        )
                    psom[m] = ps_o
                    # denominator tree on DVE (bf16 2x mode)
                    lvl = ets
                    while len(lvl) > 1:
                        nxt = []
                        for i in range(len(lvl) // 2):
                            s1 = trp.tile([P, 1024], bf16, tag="tr1",
                                          name="s1")
                            nc.vector.tensor_tensor(
                                s1[:], lvl[2 * i][:], lvl[2 * i + 1][:], ADD,
                            )
                            nxt.append(s1)
                        lvl = nxt
                    den = dnp.tile([P, 512], bf16, tag="den", name="den")
                    nc.vector.tensor_tensor(
                        den[:], lvl[0][:, 0:512], lvl[0][:, 512:1024], ADD,
                    )
                    etsm[(t, m)] = den

                def do_norm(t, m):
                    den, ps_o = etsm[(t, m)], psom[m]
                    ps_d = ppB.tile([P, 512], f32, tag="sm", name="ps_d")
                    nc.tensor.matmul(ps_d[:], ones_sb[:], den[:],
                                     start=True, stop=True)
                    rec = dnp.tile([P, 512], f32, tag="rec", name="rec")
                    nc.vector.reciprocal_approx_fast(rec[:], ps_d[:])
                    at = atp.tile([P, 512], bf16, tag="at", name="at")
                    nc.vector.tensor_tensor(at[:], ps_o[:], rec[:], MULT)
                    ats[m] = at

                do_scores(0, 0)
                do_scores(0, 1)
                for t in range(NCHUNK):
                    if t == 1:
                        prefetch_kv(1)
                    elif t == 2:
                        prefetch_q(1)
                    do_pv(t, 0)
                    do_scores(t, 2)
                    do_norm(t, 0)
                    do_pv(t, 1)
                    do_scores(t, 3)
                    do_norm(t, 1)
                    do_pv(t, 2)
                    do_norm(t, 2)
                    do_pv(t, 3)
                    do_norm(t, 3)
                    if t + 1 < NCHUNK:
                        do_scores(t + 1, 0)
                        do_scores(t + 1, 1)
                    # partial y for this token chunk over own features.
                    # For the last chunk, emit nt tiles piece-by-piece
                    # (nt%4 == 0, {1,2}, 3) and fire each piece's RS as soon
                    # as its stores are in the gpsimd queue, so only the
                    # final 1MB piece serializes after the last matmul.
                    halved = {NCHUNK - 1: (BTh, OTh)}
                    if t in halved:
                        BTt, OTt = halved[t]
                        nt_order = [nt for q0, q1 in LQ
                                    for nt in range(DIM // P)
                                    if q0 <= nt % 4 < q1]
                        piece_end = {}
                        acc = 0
                        for h, (q0, q1) in enumerate(LQ):
                            acc += 8 * (q1 - q0)
                            piece_end[acc - 1] = h
                    else:
                        nt_order = list(range(DIM // P))
                        piece_end = {}
                    for ni, nt in enumerate(nt_order):
                        ps_y = ppB.tile([P, 512], f32, tag="sm", name="ps_y")
                        for j in range(HPC):
                            nc.tensor.matmul(
                                ps_y[:], woc[:, j, nt], ats[j][:],
                                start=(j == 0), stop=(j == HPC - 1),
                            )
                        yb = ybp.tile([P, 512], bf16, tag="yb", name="yb")
                        # alternate eviction engine so neither Vector nor
                        # Scalar becomes the chunk-end bottleneck
                        if nt % 2 == 0:
                            nc.vector.tensor_copy(out=yb[:], in_=ps_y[:])
                        else:
                            nc.scalar.activation(yb[:], ps_y[:], COPY)
                        if t in halved:
                            h = next(hh for hh, (q0, q1) in enumerate(LQ)
                                     if q0 <= nt % 4 < q1)
                            q0 = LQ[h][0]
                            nc.gpsimd.dma_start(
                                BTt[h][nt // HPC]
                                   [ds(P * (nt % 4 - q0), P), :],
                                yb[:],
                            )
                            if ni in piece_end:
                                hh = piece_end[ni]
                                nc.gpsimd.collective_compute(
                                    "ReduceScatter",
                                    mybir.AluOpType.add,
                                    replica_groups=[list(range(N_CORES))],
                                    ins=[BTt[hh][:]],
                                    outs=[OTt[hh][:]],
                                )
                        else:
                            nc.gpsimd.dma_start(
                                BT[t][nt // HPC][ds(P * (nt % HPC), P), :],
                                yb[:],
                            )
                    if t not in halved:
                        nc.gpsimd.collective_compute(
                            "ReduceScatter",
                            mybir.AluOpType.add,
                            replica_groups=[list(range(N_CORES))],
                            ins=[BT[t][:]],
                            outs=[OT[t][:]],
                        )
                # all output copies at the end: the sync ring is idle then,
                # the early chunks' copies overlap the final RS pieces, and
                # no mid-phase HBM traffic competes with the collectives
                for tp in range(NCHUNK - 1):
                    nc.sync.dma_start(
                        oe3[:, :, ds(512 * tp, 512)],
                        OT[tp][:].rearrange("(dt p) q -> p dt q", p=P),
                    )
                for h, (q0, q1) in enumerate(LQ):
                    nc.sync.dma_start(
                        oe3[:, ds(q0, q1 - q0), ds(512 * (NCHUNK - 1), 512)],
                        OTh[h][:].rearrange("(dt p) q -> p dt q", p=P),
                    )

    nc.compile()
    return nc


def _prep_inputs(x, freqs_cos, freqs_sin, wq, wk, wv, wo):
    bf = ml_dtypes.bfloat16
    x = np.asarray(x, dtype=np.float32)
    fc = np.asarray(freqs_cos, dtype=np.float32)
    fs = np.asarray(freqs_sin, dtype=np.float32)
    wq = np.asarray(wq, dtype=np.float32)
    wk = np.asarray(wk, dtype=np.float32)
    wv = np.asarray(wv, dtype=np.float32)
    wo = np.asarray(wo, dtype=np.float32)

    cb = np.ascontiguousarray(np.repeat(fc.T, 2, axis=0)).astype(bf)
    ss = np.repeat(fs.T, 2, axis=0)
    ss[0::2, :] *= -1.0                      # even rows: -sin, odd rows: +sin
    ss = np.ascontiguousarray(ss).astype(bf)

    ones = np.ones((P, P), dtype=bf)

    xTf = np.ascontiguousarray(x.reshape(TOK, DIM).T).astype(bf)

    def pack_qk(w, rows):
        # [4096 in, 512 out] -> [m 4, p_in 128, ko 32, mc 128]
        wT = w[rows].T
        return np.ascontiguousarray(
            wT.reshape(KO, P, HPC, P).transpose(2, 1, 0, 3)
        ).astype(bf)

    in_maps = []
    for c in range(N_CORES):
        rows = slice(FPC * c, FPC * (c + 1))
        # wv as matmul rhs: [p_in 128, ko 32, out 512]
        wvf = np.ascontiguousarray(
            wv[rows].T.reshape(KO, P, FPC).transpose(1, 0, 2)
        ).astype(bf)
        # wo rows for this core's features: [p_feat 128, j 4, nt 32, d 128]
        woc = np.ascontiguousarray(
            wo.T[rows].reshape(HPC, P, DIM // P, P).transpose(1, 0, 2, 3)
        ).astype(bf)
        in_maps.append({
            "xT": xTf,
            "wqH": pack_qk(wq, rows),
            "wkH": pack_qk(wk, rows),
            "wvH": wvf,
            "woH": woc,
            "cb": cb,
            "ss": ss,
            "ones": ones,
        })
    return in_maps


def _gather(results):
    y = np.empty((B, S, DIM), dtype=np.float32)
    for c in range(N_CORES):
        o = np.asarray(results[c]["out"], dtype=np.float32)  # [512, 4096]
        y[:, :, FPC * c:FPC * (c + 1)] = o.T.reshape(B, S, FPC)
    return y


def kernel(x, start_pos, freqs_cos, freqs_sin, wq, wk, wv, wo, trace=False):
    if "nc" not in _CACHE:
        _CACHE["nc"] = _build()
    nc = _CACHE["nc"]
    in_maps = _prep_inputs(x, freqs_cos, freqs_sin, wq, wk, wv, wo)
    res = run_bass_kernel_spmd(
        nc, in_maps, core_ids=list(range(N_CORES)), trace=trace
    )
    _CACHE["last_result"] = res
    return _gather(res.results)
